# revision 29
# baseline (speedup 1.0000x reference)
"""Segment-mean (sorted index) Trainium2 Bass kernel — telescope edition.

Algorithm (per core, data-parallel over elements, 8 cores):
  - Core gets a contiguous shard of E elements laid out as 128 partitions x
    (E/128) contiguous elements; each partition holds rpp rows of 256 elements.
  - Contract (host-verified): index is sorted and the row-head sequence
    h[r] = idx[256*r] advances by 0 or 1 between consecutive rows, so each
    256-row spans at most 2 segments.
  - Phase A (streaming): per chunk of 8 rows, three batched DVE passes
    (scrD = (idx - cb)*x; reduce scrD -> IXS; reduce idx(i32) -> IS, exact)
    plus per-row ScalarE copy+accum -> RS (row sums).
  - Phase B (telescope): per-partition prefix RP = scan(RS).  The
    cross-partition offset off[p] = sum_{q<p} Xtot_q comes from a
    triangular-ones matmul on the idle TensorE (no DRAM bounce).  Records
        W'[h] = off + RP[rb_h] - T[rb_h],   Wc'[h] = CPg[rb_h] - C[rb_h]
    scattered as interleaved (W, Wc) pairs at the aligned slot of each
    run-end row telescope into per-segment sums/counts as adjacent pair
    differences, with no seam corrections.  One extra record per core
    (p=127 tail) rides a separate window added after differencing.
  - Assembly: interleaved window -> DRAM fold -> 2(K+1)-wide shifted views
    summed -> pair-shifted difference -> + tail window -> one indirect DMA
    into an interleaved [2*SLAB] f32 slab -> AllReduce(add) -> mean =
    sum / max(cnt,1) -> direct SBUF->DRAM output.
"""

import sys

sys.path.insert(0, "/opt/trn_rl_repo")

import numpy as np

from concourse import bacc, bass, mybir
from concourse import tile
from concourse.bass_utils import run_bass_kernel_spmd

F32 = mybir.dt.float32
I32 = mybir.dt.int32
I16 = mybir.dt.int16
U16 = mybir.dt.uint16

AX = mybir.AxisListType.X
OP = mybir.AluOpType

N_CORES = 8
P = 128
ROW = 256
NSEG = 100000
# 128 * 800; nseg = 125 * 800 exactly, so the output DMA needs no ragged
# remainder (a partition-offset SBUF source reads the wrong partition).
SLAB = 102400
WIN = 256  # window cells (pairs) per partition


def build_nc(
    epc: int,
    n_chunks: int,
    idx64: bool,
    K: int = 98,
    OFS: int = 80,
    slab: int = SLAB,
    nseg: int = NSEG,
    debug: bool = False,
):
    """Build the per-core bass program. epc = P * rpp * ROW elements."""
    assert epc % (P * ROW) == 0
    epp = epc // P
    rpp = epp // ROW
    assert rpp % n_chunks == 0
    cr = rpp // n_chunks
    assert slab % P == 0
    slabf = slab // P

    # fold geometry (in pair units; interleaved buffers use 2x cells)
    m_lo = -((WIN - OFS - 1) // K)
    m_hi = (OFS + K - 1) // K
    pitch = max(OFS - m_lo * K + K, WIN + (m_hi * K - OFS))
    pitch = ((pitch + 31) // 32) * 32
    pitch2 = 2 * pitch
    mpad = max(-m_lo, m_hi) + 1
    wf_rows = ((P + 2 * mpad + 3) // 4) * 4
    assert K * P <= slab - 64

    nc = bacc.Bacc("TRN2", target_bir_lowering=False, debug=False, num_devices=N_CORES)

    if idx64:
        idx_ext = nc.declare_dram_parameter("idx", [epc, 2], I32, isOutput=False)
    else:
        idx_ext = nc.declare_dram_parameter("idx", [epc], I32, isOutput=False)
    x_ext = nc.declare_dram_parameter("x", [epc], F32, isOutput=False)
    out_ext = nc.declare_dram_parameter("out", [nseg], F32, isOutput=True)
    if debug:
        rpp_ = epc // P // ROW
        dbg_exts = {
            "dbg_off": nc.declare_dram_parameter("dbg_off", [P, 2], F32, isOutput=True),
            "dbg_W": nc.declare_dram_parameter("dbg_W", [P, rpp_], F32, isOutput=True),
            "dbg_Wc": nc.declare_dram_parameter("dbg_Wc", [P, rpp_], F32, isOutput=True),
            "dbg_seg": nc.declare_dram_parameter("dbg_seg", [P, 2 * K], F32, isOutput=True),
            "dbg_slab": nc.declare_dram_parameter("dbg_slab", [2 * slab], F32, isOutput=True),
        }

    x_v = x_ext.ap().rearrange("(p e) -> p e", p=P)
    if idx64:
        i_v = idx_ext.ap().rearrange("(p e) w -> p e w", p=P)
    else:
        i_v = idx_ext.ap().rearrange("(p e) -> p e", p=P)

    with tile.TileContext(nc) as tc:
        with (
            tc.tile_pool(name="xs", bufs=4) as xpool,
            tc.tile_pool(name="is_", bufs=4) as ipool,
            tc.tile_pool(name="wk", bufs=2) as wkpool,
            tc.tile_pool(name="pers", bufs=1) as pp,
            tc.tile_pool(name="ps", bufs=1, space="PSUM") as psp,
            tc.tile_pool(name="dram", bufs=1, space="DRAM") as dp,
        ):
            arpad = 1024  # keep real data clear of the collective buffer tail
            slab_t = dp.tile([2 * slab + arpad], F32, tag="slab")
            ar_t = dp.tile([2 * slab + arpad], F32, tag="ar", addr_space="Shared")
            b1_t = dp.tile([P + 1, 1], I32, tag="b1")
            wfW_t = dp.tile([wf_rows, pitch2], F32, tag="wfW")
            wfX_t = dp.tile([wf_rows, pitch2], F32, tag="wfX")

            H = pp.tile([P, rpp], I32, tag="H")  # row heads
            CBr = pp.tile([P, rpp], I32, tag="CBr")  # per-row chunk base
            IXS = pp.tile([P, rpp], F32, tag="IXS")  # row sums of (idx-cb)*x
            RS = pp.tile([P, rpp], F32, tag="RS")  # row sums of x
            # half-row sums of idx: each <= 128*99999 < 2^24, exact even in
            # the DVE's internal fp32 accumulator (full rows overflow 2^24)
            ISh = pp.tile([P, 2 * rpp], I32, tag="ISh")
            XCT = pp.tile([P, 1], F32, tag="XCT")  # partition total of x

            # constants: K*p, global count prefix, strictly-lower-tri ones
            Kp = pp.tile([P, 1], I32, tag="Kp")
            nc.gpsimd.iota(Kp[:], pattern=[[0, 1]], base=0, channel_multiplier=K)
            CPg = pp.tile([P, rpp], I32, tag="CPg")
            nc.gpsimd.iota(
                CPg[:], pattern=[[ROW, rpp]], base=ROW, channel_multiplier=epp
            )
            CPgF = pp.tile([P, rpp], F32, tag="CPgF")
            nc.vector.tensor_copy(out=CPgF[:], in_=CPg[:])
            TRIr = pp.tile([P, P], I32, tag="TRIr")
            nc.gpsimd.iota(TRIr[:], pattern=[[1, P]], base=0, channel_multiplier=-1)
            tri = pp.tile([P, P], F32, tag="tri")
            nc.vector.tensor_scalar(
                out=tri[:], in0=TRIr[:], scalar1=0, scalar2=None, op0=OP.is_gt
            )

            Hnf = pp.tile([P, 1], I32, tag="Hnf")
            sent1 = pp.tile([1, 1], I32, tag="sent1")
            base0 = pp.tile([P, 1], I32, tag="base0")
            vmask = pp.tile([P, 1], F32, tag="vmask")
            sbase = pp.tile([P, 1], I32, tag="sbase")
            offs = pp.tile([P, 1], I32, tag="offs")

            # zero-fills: slab and wf guard rows. sc doubles as zero source
            # early and the all-reduce result tile at the end.
            sc = pp.tile([P, 2 * slabf], F32, tag="sc")
            nc.vector.memset(sc[:], 0)
            nc.sync.dma_start(out=slab_t[0 : 2 * slab], in_=sc[:])
            zp = pp.tile([P, arpad // P], F32, tag="zp")
            nc.vector.memset(zp[:], 0)
            nc.sync.dma_start(out=slab_t[2 * slab : 2 * slab + arpad], in_=zp[:])
            zg = pp.tile([mpad, pitch2], F32, tag="zg")
            nc.vector.memset(zg[:], 0)
            gtop = wf_rows - (mpad + P)
            for wf in (wfW_t, wfX_t):
                nc.sync.dma_start(out=wf[0:mpad, :], in_=zg[:])
                nc.sync.dma_start(out=wf[mpad + P : wf_rows, :], in_=zg[0:gtop, :])

            nc.vector.memset(XCT[:], 0)

            # ---------------- Phase A: stream chunks ----------------
            segs = [(c * cr, cr) for c in range(n_chunks)]
            for r0, nr in segs:
                sf = nr * ROW
                cs = slice(r0, r0 + nr)
                xt = xpool.tile([P, sf], F32, tag="x")
                it = ipool.tile([P, sf], I32, tag="i")
                nc.sync.dma_start(out=xt[:], in_=x_v[:, r0 * ROW : r0 * ROW + sf])
                if idx64:
                    nc.scalar.dma_start(
                        out=it[:],
                        in_=i_v[:, r0 * ROW : r0 * ROW + sf, 0:1].squeeze(axis=2),
                    )
                else:
                    nc.scalar.dma_start(
                        out=it[:], in_=i_v[:, r0 * ROW : r0 * ROW + sf]
                    )

                i3 = it[:].rearrange("p (r e) -> p r e", e=ROW)
                x3 = xt[:].rearrange("p (r e) -> p r e", e=ROW)

                nc.vector.tensor_copy(out=H[:, cs], in_=i3[:, :, 0:1].squeeze(axis=2))
                nc.vector.tensor_copy(
                    out=CBr[:, cs], in_=H[:, r0 : r0 + 1].to_broadcast([P, nr])
                )

                # batched: scrD = (idx - cb) * x ; reduce -> IXS ; reduce idx -> IS
                scrD = wkpool.tile([P, sf], F32, tag="scrD")
                nc.vector.scalar_tensor_tensor(
                    out=scrD[:],
                    in0=it[:],
                    scalar=H[:, r0 : r0 + 1],
                    in1=xt[:],
                    op0=OP.subtract,
                    op1=OP.mult,
                )
                nc.vector.tensor_reduce(
                    out=IXS[:, cs],
                    in_=scrD[:].rearrange("p (r e) -> p r e", e=ROW),
                    axis=AX,
                    op=OP.add,
                )
                with nc.allow_low_precision(reason="half-row sums stay < 2^24"):
                    nc.vector.tensor_reduce(
                        out=ISh[:, 2 * r0 : 2 * (r0 + nr)],
                        in_=it[:].rearrange("p (r e) -> p r e", e=ROW // 2),
                        axis=AX,
                        op=OP.add,
                    )

                # ScalarE: per-row sums of x
                scrA = wkpool.tile([P, ROW], F32, tag="scrA", bufs=3)
                for r in range(nr):
                    g = r0 + r
                    nc.scalar.activation(
                        out=scrA[:],
                        in_=x3[:, r],
                        func=mybir.ActivationFunctionType.Copy,
                        accum_out=RS[:, g : g + 1],
                    )
                # running partition totals of x (feeds the offset matmul)
                ct = wkpool.tile([P, 1], F32, tag="ct", bufs=3)
                nc.vector.tensor_reduce(out=ct[:], in_=RS[:, cs], axis=AX, op=OP.add)
                nc.vector.tensor_tensor(out=XCT[:], in0=XCT[:], in1=ct[:], op=OP.add)

                if r0 == cr:  # after first chunk: seam bounce for Hnf/base0
                    nc.vector.memset(sent1[:], -1)
                    nc.sync.dma_start(out=b1_t[0:P, :], in_=H[:, 0:1])
                    nc.sync.dma_start(out=b1_t[P : P + 1, :], in_=sent1[:])
                    nc.sync.dma_start(out=Hnf[:], in_=b1_t[1 : P + 1, :])
                    nc.sync.dma_start(
                        out=base0[:], in_=b1_t[0:1, 0:1].to_broadcast([P, 1])
                    )
                    nc.vector.tensor_scalar(
                        out=vmask[:], in0=Hnf[:], scalar1=-1, scalar2=None,
                        op0=OP.is_equal,
                    )
                    nc.vector.tensor_tensor(
                        out=sbase[:], in0=base0[:], in1=Kp[:], op=OP.add
                    )
                    nc.vector.tensor_copy(out=offs[:], in_=sbase[:])
                    nc.vector.tensor_scalar(
                        out=sbase[:], in0=sbase[:], scalar1=-OFS, scalar2=None,
                        op0=OP.add,
                    )

            # ---------------- cross-partition offset (TensorE) ----------------
            offp = psp.tile([P, 1], F32, tag="offp")
            nc.tensor.matmul(offp[:], tri[:], XCT[:], start=True, stop=True)
            off = pp.tile([P, 1], F32, tag="off")
            nc.vector.tensor_copy(out=off[:], in_=offp[:])

            # ---------------- Phase B: telescope records ----------------
            hpF = pp.tile([P, rpp], F32, tag="hpF")
            nc.vector.tensor_tensor(out=hpF[:], in0=H[:], in1=CBr[:], op=OP.subtract)
            t1 = pp.tile([P, rpp], F32, tag="t1")
            nc.vector.tensor_tensor(out=t1[:], in0=hpF[:], in1=RS[:], op=OP.mult)
            T = pp.tile([P, rpp], F32, tag="T")
            nc.vector.tensor_tensor(out=T[:], in0=IXS[:], in1=t1[:], op=OP.subtract)

            # Ci = sum(idx) - 256*H via half-rows: every intermediate stays
            # < 2^24 so the math is exact regardless of ALU precision
            t2i = pp.tile([P, rpp], I32, tag="t2i")
            nc.vector.tensor_scalar(
                out=t2i[:], in0=H[:], scalar1=7, scalar2=None,
                op0=OP.arith_shift_left,
            )
            IS3 = ISh[:].rearrange("p (r w) -> p r w", w=2)
            Ch0 = pp.tile([P, rpp], I32, tag="Ch0")
            Ch1 = pp.tile([P, rpp], I32, tag="Ch1")
            nc.vector.tensor_tensor(
                out=Ch0[:], in0=IS3[:, :, 0:1].squeeze(axis=2), in1=t2i[:],
                op=OP.subtract,
            )
            nc.vector.tensor_tensor(
                out=Ch1[:], in0=IS3[:, :, 1:2].squeeze(axis=2), in1=t2i[:],
                op=OP.subtract,
            )
            Ci = pp.tile([P, rpp], I32, tag="Ci")
            nc.vector.tensor_tensor(out=Ci[:], in0=Ch0[:], in1=Ch1[:], op=OP.add)
            CiF = pp.tile([P, rpp], F32, tag="CiF")
            nc.vector.tensor_copy(out=CiF[:], in_=Ci[:])

            # RP prefix of row sums; W = RP - T + off ; Wc = CPg - Ci
            onesR = pp.tile([P, rpp], F32, tag="onesR")
            nc.vector.memset(onesR[:], 1.0)
            RP = pp.tile([P, rpp], F32, tag="RP")
            nc.vector.tensor_tensor_scan(
                out=RP[:], data0=onesR[:], data1=RS[:], initial=0.0,
                op0=OP.mult, op1=OP.add,
            )
            W = pp.tile([P, rpp], F32, tag="W")
            nc.vector.tensor_tensor(out=W[:], in0=RP[:], in1=T[:], op=OP.subtract)
            nc.vector.tensor_tensor(
                out=W[:], in0=W[:], in1=off[:].to_broadcast([P, rpp]), op=OP.add
            )
            Wc = pp.tile([P, rpp], F32, tag="Wc")
            nc.vector.tensor_tensor(out=Wc[:], in0=CPgF[:], in1=CiF[:], op=OP.subtract)

            # interleaved record data: (W, Wc) pairs
            data2 = pp.tile([P, 2 * rpp], F32, tag="data2")
            d3 = data2[:].rearrange("p (r w) -> p r w", w=2)
            nc.vector.tensor_copy(out=d3[:, :, 0:1].squeeze(axis=2), in_=W[:])
            nc.vector.tensor_copy(out=d3[:, :, 1:2].squeeze(axis=2), in_=Wc[:])

            # last-of-run mask with partition-seam suppression at col rpp-1
            lastm = pp.tile([P, rpp], F32, tag="lastm")
            nc.vector.tensor_tensor(
                out=lastm[:, : rpp - 1], in0=H[:, : rpp - 1], in1=H[:, 1:],
                op=OP.not_equal,
            )
            nc.vector.tensor_tensor(
                out=lastm[:, rpp - 1 : rpp], in0=H[:, rpp - 1 : rpp], in1=Hnf[:],
                op=OP.not_equal,
            )

            # aligned slots: slot = H - base0 - K*p + OFS
            slotf = pp.tile([P, rpp], F32, tag="slotf")
            nc.vector.tensor_tensor(
                out=slotf[:], in0=H[:],
                in1=sbase[:].to_broadcast([P, rpp]), op=OP.subtract,
            )

            # u16 quad indices: record r -> cells (4s, 4s+1, 4s+2, 4s+3)
            idxAf = pp.tile([P, rpp], F32, tag="idxAf")
            nc.vector.tensor_scalar(
                out=idxAf[:], in0=slotf[:], scalar1=1.0, scalar2=None, op0=OP.add
            )
            nc.vector.tensor_tensor(out=idxAf[:], in0=idxAf[:], in1=lastm[:], op=OP.mult)
            nc.vector.tensor_scalar(
                out=idxAf[:], in0=idxAf[:], scalar1=4.0, scalar2=-4.0,
                op0=OP.mult, op1=OP.add,
            )  # = 4*slot if last else -4
            pidxf = pp.tile([P, 4 * rpp], F32, tag="pidxf")
            p4 = pidxf[:].rearrange("p (r w) -> p r w", w=4)
            tq = pp.tile([P, rpp], F32, tag="tq")
            nc.vector.tensor_copy(out=p4[:, :, 0:1].squeeze(axis=2), in_=idxAf[:])
            nc.vector.tensor_scalar(
                out=tq[:], in0=idxAf[:], scalar1=1.0, scalar2=None, op0=OP.add
            )
            nc.vector.tensor_copy(out=p4[:, :, 1:2].squeeze(axis=2), in_=tq[:])
            nc.vector.tensor_scalar(
                out=tq[:], in0=tq[:], scalar1=1.0, scalar2=None, op0=OP.add
            )
            nc.vector.tensor_copy(out=p4[:, :, 2:3].squeeze(axis=2), in_=tq[:])
            nc.vector.tensor_scalar(
                out=tq[:], in0=tq[:], scalar1=1.0, scalar2=None, op0=OP.add
            )
            nc.vector.tensor_copy(out=p4[:, :, 3:4].squeeze(axis=2), in_=tq[:])
            pidx16 = pp.tile([P, 4 * rpp], I16, tag="pidx16")
            nc.vector.tensor_copy(out=pidx16[:], in_=pidxf[:])

            # tail record (p=127 only): pair (off+XCT, CPg_last) at slot_last+1
            dataX = pp.tile([P, 2], F32, tag="dataX")
            nc.vector.tensor_tensor(
                out=dataX[:, 0:1], in0=off[:], in1=XCT[:], op=OP.add
            )
            nc.vector.tensor_copy(out=dataX[:, 1:2], in_=CPgF[:, rpp - 1 : rpp])
            pidxXf = pp.tile([P, 4], F32, tag="pidxXf")
            u2 = pp.tile([P, 1], F32, tag="u2")
            nc.vector.tensor_scalar(
                out=u2[:], in0=slotf[:, rpp - 1 : rpp],
                scalar1=4.0, scalar2=4.0, op0=OP.mult, op1=OP.add,
            )  # 4*(slot+1)
            for w in range(4):
                nc.vector.tensor_scalar(
                    out=pidxXf[:, w : w + 1], in0=u2[:], scalar1=float(w),
                    scalar2=None, op0=OP.add,
                )
            # mask quads: vmask*(val+1) - 1
            nc.vector.tensor_scalar(
                out=pidxXf[:], in0=pidxXf[:], scalar1=1.0, scalar2=None, op0=OP.add
            )
            nc.vector.tensor_tensor(
                out=pidxXf[:], in0=pidxXf[:],
                in1=vmask[:].to_broadcast([P, 4]), op=OP.mult,
            )
            nc.vector.tensor_scalar(
                out=pidxXf[:], in0=pidxXf[:], scalar1=-1.0, scalar2=None, op0=OP.add
            )
            pidxX16 = pp.tile([P, 4], I16, tag="pidxX16")
            nc.vector.tensor_copy(out=pidxX16[:], in_=pidxXf[:])

            # local scatters into interleaved windows (zero-filled by the op)
            winW = pp.tile([P, pitch2], F32, tag="winW")
            winX = pp.tile([P, pitch2], F32, tag="winX")
            nc.gpsimd.local_scatter(
                out_ap=winW[:].bitcast(U16),
                data_ap=data2[:].bitcast(U16),
                idxs_ap=pidx16[:, 0 : 4 * rpp],
                channels=P, num_elems=2 * pitch2, num_idxs=4 * rpp,
            )
            nc.gpsimd.local_scatter(
                out_ap=winX[:].bitcast(U16),
                data_ap=dataX[:].bitcast(U16),
                idxs_ap=pidxX16[:, 0:4],
                channels=P, num_elems=2 * pitch2, num_idxs=4,
            )

            # ---------------- fold assembly ----------------
            nc.sync.dma_start(out=wfW_t[mpad : mpad + P, :], in_=winW[:])
            nc.sync.dma_start(out=wfX_t[mpad : mpad + P, :], in_=winX[:])

            accWe = pp.tile([P, 2 * K + 2], F32, tag="accWe")
            accX = pp.tile([P, 2 * K], F32, tag="accX")
            for wf_t, acc, wid, sh in ((wfW_t, accWe, 2 * K + 2, 2), (wfX_t, accX, 2 * K, 0)):
                wf_f = wf_t[:].rearrange("a b -> (a b)")
                first = True
                for m in range(m_lo, m_hi + 1):
                    src0 = (mpad + m) * pitch2 + 2 * (OFS - m * K) - sh
                    assert src0 >= 0 and src0 + (P - 1) * pitch2 + wid <= wf_rows * pitch2
                    view = wf_f[src0 : src0 + P * pitch2].rearrange(
                        "(p b) -> p b", b=pitch2
                    )[:, 0:wid]
                    vtile = pp.tile([P, wid], F32, tag=f"vt{wid}", bufs=4)
                    nc.sync.dma_start(out=vtile[:], in_=view)
                    if first:
                        nc.vector.tensor_copy(out=acc[:], in_=vtile[:])
                        first = False
                    else:
                        nc.vector.tensor_tensor(
                            out=acc[:], in0=acc[:], in1=vtile[:], op=OP.add
                        )

            # pair-shifted telescope difference + tail
            seg2 = pp.tile([P, 2 * K], F32, tag="seg2")
            nc.vector.tensor_tensor(
                out=seg2[:], in0=accWe[:, 2 : 2 * K + 2], in1=accWe[:, 0 : 2 * K],
                op=OP.subtract,
            )
            nc.vector.tensor_tensor(out=seg2[:], in0=seg2[:], in1=accX[:], op=OP.add)

            if debug:
                nc.sync.dma_start(out=dbg_exts["dbg_off"].ap()[:, 0:1], in_=off[:])
                nc.sync.dma_start(out=dbg_exts["dbg_off"].ap()[:, 1:2], in_=XCT[:])
                nc.sync.dma_start(out=dbg_exts["dbg_W"].ap(), in_=W[:])
                nc.sync.dma_start(out=dbg_exts["dbg_Wc"].ap(), in_=Wc[:])
                nc.sync.dma_start(out=dbg_exts["dbg_seg"].ap(), in_=seg2[:])

            # de-interleave (sum, cnt) pairs; per-partition indirect offsets
            # must stay small (hw offset-field limit) so counts ride the
            # full-width element_offset instead of doubled offsets.
            segS = pp.tile([P, K], F32, tag="segS")
            segC = pp.tile([P, K], F32, tag="segC")
            sg3 = seg2[:].rearrange("p (k w) -> p k w", w=2)
            nc.vector.tensor_copy(out=segS[:], in_=sg3[:, :, 0:1].squeeze(axis=2))
            nc.vector.tensor_copy(out=segC[:], in_=sg3[:, :, 1:2].squeeze(axis=2))

            # ---------------- disjoint indirect placement --------
            slab_2d = slab_t[:].rearrange("(a b) -> a b", b=1)
            nc.gpsimd.indirect_dma_start(
                out=slab_2d,
                out_offset=bass.IndirectOffsetOnAxis(ap=offs[:, 0:1], axis=0),
                in_=segS[:],
                in_offset=None,
            )
            nc.gpsimd.indirect_dma_start(
                out=slab_2d,
                out_offset=bass.IndirectOffsetOnAxis(ap=offs[:, 0:1], axis=0),
                in_=segC[:],
                in_offset=None,
                element_offset=slab,
            )

            # ---------------- all-reduce + divide ----------------
            nc.gpsimd.collective_compute(
                "AllReduce",
                OP.add,
                replica_groups=[list(range(N_CORES))],
                ins=[slab_t[:].opt()],
                outs=[ar_t[:].opt()],
            )
            nc.sync.dma_start(
                out=sc[:].rearrange("p (h e) -> p h e", h=2),
                in_=ar_t[0 : 2 * slab].rearrange("(h p e) -> p h e", h=2, p=P),
            )
            if debug:
                nc.sync.dma_start(out=dbg_exts["dbg_slab"].ap(), in_=slab_t[0 : 2 * slab])
            sums_v = sc[:, 0:slabf]
            cnts_v = sc[:, slabf : 2 * slabf]
            cntsF = pp.tile([P, slabf], F32, tag="cntsF")
            nc.vector.tensor_scalar(
                out=cntsF[:], in0=cnts_v, scalar1=1.0, scalar2=None, op0=OP.max
            )
            nc.vector.reciprocal(out=cntsF[:], in_=cntsF[:])
            meanF = pp.tile([P, slabf], F32, tag="meanF")
            nc.vector.tensor_tensor(
                out=meanF[:], in0=sums_v, in1=cntsF[:], op=OP.mult
            )
            # direct output DMA (no DRAM hop); nseg = nfull * slabf exactly
            nfull = nseg // slabf
            assert nfull * slabf == nseg, (nfull, slabf, nseg)
            nc.sync.dma_start(
                out=out_ext.ap().rearrange("(p e) -> p e", p=nfull),
                in_=meanF[0:nfull, :],
            )

    nc.finalize()
    return nc


_NC_CACHE: dict = {}


def _get_nc(*key):
    if key not in _NC_CACHE:
        _NC_CACHE[key] = build_nc(*key)
    return _NC_CACHE[key]


def kernel(x: np.ndarray, index: np.ndarray) -> np.ndarray:
    n = x.shape[0]
    assert n % (N_CORES * P * ROW) == 0, n
    epc = n // N_CORES
    idx64 = index.dtype == np.int64
    K, OFS = 98, 80
    # cheap structural check on row heads (the algorithm's contract)
    heads = np.ascontiguousarray(index[::ROW]).astype(np.int64)
    dh = np.diff(heads)
    if dh.min() < 0 or dh.max() > 1:
        raise ValueError("row-head steps outside {0,1}; kernel contract violated")
    hc = heads.reshape(N_CORES, P, -1)
    slot = hc - hc[:, 0:1, 0:1] - K * np.arange(P)[None, :, None] + OFS
    if slot.min() < 0 or slot.max() + 1 >= WIN:
        raise ValueError("alignment window overflow; adjust K/OFS")
    # tail position must stay within the strip-covered slab region
    tail_pos = hc[:, -1, -1] + 1 - hc[:, 0, 0]
    if (tail_pos >= 128 * K).any():
        raise ValueError("core tail beyond strip coverage; adjust K")

    nc = _get_nc(epc, 16, idx64, K, OFS, SLAB, NSEG)

    in_maps = []
    for c in range(N_CORES):
        xs = np.ascontiguousarray(x[c * epc : (c + 1) * epc], dtype=np.float32)
        ish = index[c * epc : (c + 1) * epc]
        if idx64:
            ii = np.ascontiguousarray(ish).view(np.int32).reshape(epc, 2)
        else:
            ii = np.ascontiguousarray(ish, dtype=np.int32)
        in_maps.append({"x": xs, "idx": ii})

    res = run_bass_kernel_spmd(
        nc, in_maps, core_ids=list(range(N_CORES)), trace=TRACE, **RUN_KWARGS
    )
    global LAST_RESULT
    LAST_RESULT = res
    out = res.results[0]["out"]
    return np.asarray(out, dtype=np.float32).ravel()


TRACE = False
RUN_KWARGS: dict = {}
LAST_RESULT = None
